# revision 1
# baseline (speedup 1.0000x reference)
"""CrossEncoderGNN (2x GIN layer + sum-pool + MLP + sigmoid) on 8 trn2 NeuronCores.

Strategy
--------
Math: GIN layer  h' = (h + A h) @ W + b  ==  (I + A) (h @ W) + b   (A acts on
rows, W on columns, so they commute).  Per layer:
  phase A: y = h @ W computed on each core for its 2500-node shard (dense
           matmul, xbar-transpose DMA provides h^T tiles as lhsT).
  AllGather: y shards (f16) -> full padded table [8*2560, 512] on every core.
  phase B: per dst-tile of 128 nodes, dma_gather the y rows of all incident
           edges (dst-sorted, self-loops included) and segment-sum them with a
           one-hot [128e x 128d] matmul into PSUM; add bias.
Pooling (graph segment-sum) is one more one-hot matmul accumulated over the
core's 20 node tiles; partial pooled [64,512] is AllReduced, and the tiny
classifier MLP + sigmoid runs replicated on every core.

Sharding: nodes (and their incident in-edges) are split 8 ways by contiguous
dst ranges: core c owns nodes [2500c, 2500c+2500), padded to 2560 rows so
every core has 20 uniform tiles of 128.
"""

import sys

for _p in ("/opt/trn_rl_repo", "/root/.axon_site/_ro/trn_rl_repo"):
    if _p not in sys.path:
        sys.path.insert(0, _p)

import os
import numpy as np
import ml_dtypes

import concourse.bass as bass
import concourse.bacc as bacc
import concourse.tile as tile
from concourse import mybir
from concourse.bass_utils import run_bass_kernel_spmd
from concourse.masks import make_identity

F16 = np.float16

N_NODES = 20000
N_EDGES = 320000
D = 512
N_GRAPHS = 64
N_CORES = 8
ROWS = N_NODES // N_CORES          # 2500 real rows per core
P = 128
TILES = (ROWS + P - 1) // P        # 20
PAD_ROWS = TILES * P               # 2560 padded rows per core
FULL_PAD = PAD_ROWS * N_CORES      # 20480
KCH = D // P                       # 4 contraction chunks of 128

LAST_EXEC_NS = None
LAST_RESULTS = None

_prog_cache = {}


HALF = PAD_ROWS // 2               # 1280 local rows per AllGather half
FULL_HALF = HALF * N_CORES         # 10240 rows per half table


GRP = TILES                        # tiles per phase-B stream (one stream per half)
N_GRP = 1
B_LEAD = 3                         # half-A streams in flight before first half-B


def _gather_split(k_max):
    """Split k_max chunks into balanced calls of <=8 chunks each."""
    n_calls = max(1, (k_max + 7) // 8)
    base = k_max // n_calls
    rem = k_max - base * n_calls
    return [base + (1 if i < rem else 0) for i in range(n_calls)]


def _derive(n_arr):
    """Shared (host+program) phase-B stream structure from the per-(tile,
    half) equalized row counts n_arr [TILES, 2].

    Phase B processes 5 groups of 4 dst tiles; within a group, first the
    half-A stream (sources in AllGather half A), then half-B. A stream is
    the concatenation of the group's 4 tile segments (n_arr rows each),
    cut into 128-row chunks (segments are not chunk-aligned; a chunk can
    straddle two tiles and then needs two S blocks / matmuls).

    Returns per (group, half) dicts with: seg_off[5], K (chunks), sizes
    (gather call split), chunk_t0/chunk_t1 (first/last tile per chunk),
    block_base (S-block index of each chunk's first block), and the global
    chunk_off / block_off of the stream.
    """
    gh = {}
    chunk_off = 0
    block_off = 0
    for g in range(N_GRP):
        for h in range(2):
            segs = [int(n_arr[g * GRP + tt, h]) for tt in range(GRP)]
            seg_off = np.concatenate([[0], np.cumsum(segs)])
            R = int(seg_off[-1])
            K = (R + P - 1) // P
            sizes = _gather_split(K)
            ks = np.arange(K)
            chunk_t0 = np.searchsorted(seg_off, ks * P, side="right") - 1
            chunk_t1 = np.minimum(
                np.searchsorted(seg_off, np.minimum(ks * P + P - 1, R - 1),
                                side="right") - 1,
                GRP - 1,
            )
            nblocks = chunk_t1 - chunk_t0 + 1
            block_base = np.concatenate([[0], np.cumsum(nblocks)])
            gh[(g, h)] = dict(
                seg_off=seg_off, R=R, K=K, sizes=sizes,
                chunk_t0=chunk_t0, chunk_t1=chunk_t1,
                block_base=block_base, chunk_off=chunk_off,
                block_off=block_off,
            )
            chunk_off += K
            block_off += int(block_base[-1])
    return gh, chunk_off, block_off


def _build_program(n_key):
    n_arr = np.asarray(n_key, np.int64).reshape(TILES, 2)
    gh, total_chunks, total_blocks = _derive(n_arr)
    kg_max = max(max(s["sizes"]) for s in gh.values())
    f32 = mybir.dt.float32
    f16 = mybir.dt.float16
    i16 = mybir.dt.int16

    nc = bacc.Bacc("TRN2", debug=False, num_devices=N_CORES, num_swdge_queues=4)

    # ---- I/O ----
    x_sh = nc.dram_tensor("x_sh", [PAD_ROWS, D], f16, kind="ExternalInput")
    idx_all = nc.dram_tensor("idx_all", [P, total_chunks * 8], i16, kind="ExternalInput")
    s_all = nc.dram_tensor("s_all", [P, total_blocks * P], f16, kind="ExternalInput")
    p_all = nc.dram_tensor("p_all", [P, TILES * N_GRAPHS], f16, kind="ExternalInput")
    w1_in = nc.dram_tensor("w1", [P, KCH * D], f16, kind="ExternalInput")
    w2_in = nc.dram_tensor("w2", [P, KCH * D], f16, kind="ExternalInput")
    b1_in = nc.dram_tensor("b1b", [P, D], f32, kind="ExternalInput")
    b2_in = nc.dram_tensor("b2b", [P, D], f32, kind="ExternalInput")
    wc1_in = nc.dram_tensor("wc1", [P, KCH * 2 * P], f32, kind="ExternalInput")
    bc1_in = nc.dram_tensor("bc1", [P, 2], f32, kind="ExternalInput")
    wc2_in = nc.dram_tensor("wc2", [P, 2], f32, kind="ExternalInput")
    bc2_in = nc.dram_tensor("bc2", [1, 1], f32, kind="ExternalInput")
    scores = nc.dram_tensor("scores", [1, N_GRAPHS], f32, kind="ExternalOutput")

    # ---- internal DRAM ----
    # y shards are split into half tensors so each AllGather half depends
    # only on the phase-A writes it actually needs.
    y1_shA = nc.dram_tensor("y1_shA", [HALF, D], f16)
    y1_shB = nc.dram_tensor("y1_shB", [HALF, D], f16)
    y2_shA = nc.dram_tensor("y2_shA", [HALF, D], f16)
    y2_shB = nc.dram_tensor("y2_shB", [HALF, D], f16)
    par1 = nc.dram_tensor("par1", [PAD_ROWS, D], f16)
    par2 = nc.dram_tensor("par2", [PAD_ROWS, D], f16)
    y1_fa = nc.dram_tensor("y1_fa", [FULL_HALF, D], f16, addr_space="Shared")
    y1_fb = nc.dram_tensor("y1_fb", [FULL_HALF, D], f16, addr_space="Shared")
    y2_fa = nc.dram_tensor("y2_fa", [FULL_HALF, D], f16, addr_space="Shared")
    y2_fb = nc.dram_tensor("y2_fb", [FULL_HALF, D], f16, addr_space="Shared")
    pool_in = nc.dram_tensor("pool_in", [N_GRAPHS, D], f32)
    pool_out = nc.dram_tensor("pool_out", [N_GRAPHS, D], f32, addr_space="Shared")

    rg = [list(range(N_CORES))]

    with tile.TileContext(nc) as tc:
        with (
            tc.tile_pool(name="const", bufs=1) as const,
            tc.tile_pool(name="xT", bufs=1) as xT_pool,
            tc.tile_pool(name="gbuf", bufs=6) as gpool,
            tc.tile_pool(name="stage", bufs=3) as stage_pool,
            tc.tile_pool(name="h2p", bufs=4) as h2_pool,
            tc.tile_pool(name="mlp", bufs=1) as mlp_pool,
            tc.tile_pool(name="psA", bufs=7, space="PSUM") as psA,
            tc.tile_pool(name="psPool", bufs=1, space="PSUM") as psPool,
        ):
            def load_xT(h_dram):
                xT = xT_pool.tile([P, KCH, PAD_ROWS], f16, tag="xT")
                for j in range(KCH):
                    nc.sync.dma_start(
                        out=xT[:, j, :],
                        in_=h_dram[:, j * P : (j + 1) * P],
                        transpose=True,
                    )
                return xT

            # Layer-1 transposes first: xbar-mode DMAs serialize against
            # normal DMAs, so issue all four before any other traffic.
            xT1 = load_xT(x_sh)

            # ---- resident constants ----
            # Bulk loads go through the ACT HWDGE ring (nc.scalar) so they
            # don't serialize with phase A's xbar transposes on the SP ring.
            idx_sb = const.tile([P, total_chunks * 8], i16)
            nc.gpsimd.dma_start(out=idx_sb[:], in_=idx_all[:])
            s_flat = const.tile([P, total_blocks * P], f16)
            nc.gpsimd.dma_start(out=s_flat[:], in_=s_all[:])
            s_sb = s_flat[:].rearrange("p (c d) -> p c d", d=P)
            p_flat = const.tile([P, TILES * N_GRAPHS], f16)
            nc.gpsimd.dma_start(out=p_flat[:], in_=p_all[:])
            p_sb = p_flat[:].rearrange("p (t g) -> p t g", g=N_GRAPHS)
            w_sb = []
            for w_in in (w1_in, w2_in):
                wt = const.tile([P, KCH * D], f16)
                nc.scalar.dma_start(out=wt[:], in_=w_in[:])
                w_sb.append(wt[:].rearrange("p (j d) -> p j d", d=D))
            b_sb = []
            for b_in in (b1_in, b2_in):
                bt = const.tile([P, D], f32)
                nc.scalar.dma_start(out=bt[:], in_=b_in[:])
                b_sb.append(bt)
            wc1_sb = const.tile([P, KCH * 2 * P], f32)
            nc.scalar.dma_start(out=wc1_sb[:], in_=wc1_in[:])
            wc1_v = wc1_sb[:].rearrange("p (j c m) -> p j c m", c=2, m=P)
            bc1_sb = const.tile([P, 2], f32)
            nc.scalar.dma_start(out=bc1_sb[:], in_=bc1_in[:])
            wc2_sb = const.tile([P, 2], f32)
            nc.scalar.dma_start(out=wc2_sb[:], in_=wc2_in[:])
            bc2_sb = const.tile([1, 1], f32)
            nc.scalar.dma_start(out=bc2_sb[:], in_=bc2_in[:])
            ident = const.tile([P, P], f32)
            make_identity(nc, ident[:])
            ident16 = const.tile([P, P], f16)
            make_identity(nc, ident16[:])

            def phase_a(h_dram, w_view, y_shA, y_shB, xT=None):
                """y = h @ W for this core's 20 row tiles; tiles 0-9 land in
                y_shA, 10-19 in y_shB (stage-written per 5 tiles)."""
                if xT is None:
                    xT = load_xT(h_dram)
                va = y_shA.ap().rearrange("(t p) d -> p t d", p=P)
                vb = y_shB.ap().rearrange("(t p) d -> p t d", p=P)
                for grp in range(4):
                    st = stage_pool.tile([P, 5, D], f16, tag="stage")
                    for tt in range(5):
                        t = grp * 5 + tt
                        ps = psA.tile([P, D], f32, tag="psA")
                        for j in range(KCH):
                            nc.tensor.matmul(
                                out=ps[:],
                                lhsT=xT[:, j, t * P : (t + 1) * P],
                                rhs=w_view[:, j, :],
                                start=(j == 0),
                                stop=(j == KCH - 1),
                            )
                        nc.vector.tensor_copy(out=st[:, tt, :], in_=ps[:])
                    view = va if grp < 2 else vb
                    c0 = (grp % 2) * 5
                    nc.sync.dma_start(
                        out=view[:, c0 : c0 + 5, :], in_=st[:]
                    )

            qn_counter = [0]

            def stream_pass(h, y_half, on_tile_done):
                """Gathers + segment matmuls for the half-h stream (all 20
                tile segments); calls on_tile_done(t, ps) at each tile's
                last block."""
                info = gh[(0, h)]
                K, sizes = info["K"], info["sizes"]
                t0s, t1s = info["chunk_t0"], info["chunk_t1"]
                bbase = info["block_base"]
                boff = info["block_off"]
                coff = info["chunk_off"]
                pss = {}
                k = 0
                for sz in sizes:
                    gt = gpool.tile([P, kg_max, D], f16, tag="g")
                    col0 = (coff + k) * 8
                    nc.gpsimd.dma_gather(
                        out_ap=gt[:, :sz, :],
                        in_ap=y_half[:],
                        idxs_ap=idx_sb[:, col0 : col0 + sz * 8],
                        num_idxs=sz * P,
                        num_idxs_reg=sz * P,
                        elem_size=D,
                        queue_num=qn_counter[0] % 4,
                    )
                    qn_counter[0] += 1
                    for kk in range(sz):
                        kc = k + kk
                        for t in range(int(t0s[kc]), int(t1s[kc]) + 1):
                            first = t not in pss
                            if first:
                                pss[t] = psA.tile(
                                    [P, D], f32, tag="psA", name=f"aggps{h}_{t}")
                            blk = boff + int(bbase[kc]) + (t - int(t0s[kc]))
                            last = (t < int(t1s[kc]) or kc == K - 1
                                    or int(t0s[kc + 1]) > t)
                            nc.tensor.matmul(
                                out=pss[t][:],
                                lhsT=s_sb[:, blk, :],
                                rhs=gt[:, kk, :],
                                start=first,
                                stop=last,
                                skip_group_check=True,
                            )
                            if last:
                                on_tile_done(t, pss.pop(t))
                    k += sz

            def pass_a(y_fa, par_dram):
                """Half-A pass: segment-sum the half-A sources of every tile
                and park the fp16 partials in DRAM."""
                par_view = par_dram.ap().rearrange("(t p) d -> p t d", p=P)
                state = {}

                def done(t, ps):
                    if t % 5 == 0:
                        state["st"] = stage_pool.tile(
                            [P, 5, D], f16, tag="stage", name=f"stpar{t}")
                    nc.vector.tensor_copy(out=state["st"][:, t % 5, :], in_=ps[:])
                    if t % 5 == 4:
                        nc.sync.dma_start(
                            out=par_view[:, t - 4 : t + 1, :], in_=state["st"][:])

                stream_pass(0, y_fa, done)

            def pass_b(y_fb, par_dram, y_shA, y_shB, b_bias, consumer):
                """Half-B pass: finish each tile's aggregate (half-B sources
                + DRAM partial + own rows + bias) and hand the fp16 result
                to consumer(t, h_tile)."""
                par_view = par_dram.ap().rearrange("(t p) d -> p t d", p=P)
                part = {}
                yown = {}

                def prefetch(t):
                    if t >= TILES:
                        return
                    pt = h2_pool.tile([P, D], f16, tag="part", name=f"part{t}")
                    nc.sync.dma_start(out=pt[:], in_=par_view[:, t, :])
                    yo = h2_pool.tile([P, D], f16, tag="yown", name=f"yown{t}")
                    src = (y_shA[t * P : (t + 1) * P, :] if t < TILES // 2
                           else y_shB[(t - TILES // 2) * P : (t - TILES // 2 + 1) * P, :])
                    nc.sync.dma_start(out=yo[:], in_=src)
                    part[t], yown[t] = pt, yo

                for t0 in range(3):
                    prefetch(t0)

                def done(t, ps):
                    hb = h2_pool.tile([P, D], f16, tag="hb", name=f"hb{t}")
                    nc.vector.tensor_add(out=hb[:], in0=ps[:], in1=b_bias[:])
                    nc.vector.tensor_add(out=hb[:], in0=hb[:], in1=part.pop(t)[:])
                    nc.vector.tensor_add(out=hb[:], in0=hb[:], in1=yown.pop(t)[:])
                    prefetch(t + 3)
                    consumer(t, hb)

                stream_pass(1, y_fb, done)

            def allgather_halves(y_shA, y_shB, y_fa, y_fb):
                nc.gpsimd.collective_compute(
                    "AllGather", mybir.AluOpType.bypass, replica_groups=rg,
                    ins=[y_shA[:]], outs=[y_fa[:]],
                )
                nc.gpsimd.collective_compute(
                    "AllGather", mybir.AluOpType.bypass, replica_groups=rg,
                    ins=[y_shB[:]], outs=[y_fb[:]],
                )

            # ---- layer 1 ----
            phase_a(x_sh, w_sb[0], y1_shA, y1_shB, xT=xT1)
            allgather_halves(y1_shA, y1_shB, y1_fa, y1_fb)
            pass_a(y1_fa, par1)

            # Layer-1 pass B fuses layer-2's dense matmul: each finished h1
            # tile is PE-transposed in SBUF and y2 = h1 @ W2 computed right
            # away, so y2's AllGather starts while layer-1 aggregation is
            # still draining (no h1 DRAM round-trip, no xbar DMA).
            h1T = xT_pool.tile([P, KCH, PAD_ROWS], f16, tag="xT", name="h1T")
            vy2a = y2_shA.ap().rearrange("(t p) d -> p t d", p=P)
            vy2b = y2_shB.ap().rearrange("(t p) d -> p t d", p=P)
            stY = {}

            def b1_consumer(t, hb):
                trp = psA.tile([P, KCH * P], f16, tag="psA", name=f"trp{t}")
                for j in range(KCH):
                    nc.tensor.transpose(
                        out=trp[:, j * P : (j + 1) * P],
                        in_=hb[:, j * P : (j + 1) * P],
                        identity=ident16[:],
                    )
                for j in range(KCH):
                    nc.vector.tensor_copy(
                        out=h1T[:, j, t * P : (t + 1) * P],
                        in_=trp[:, j * P : (j + 1) * P],
                    )
                y2ps = psA.tile([P, D], f32, tag="psA", name=f"y2ps{t}")
                for j in range(KCH):
                    nc.tensor.matmul(
                        out=y2ps[:],
                        lhsT=h1T[:, j, t * P : (t + 1) * P],
                        rhs=w_sb[1][:, j, :],
                        start=(j == 0),
                        stop=(j == KCH - 1),
                    )
                if t % 5 == 0:
                    stY["st"] = stage_pool.tile(
                        [P, 5, D], f16, tag="stage", name=f"sty{t}")
                nc.vector.tensor_copy(out=stY["st"][:, t % 5, :], in_=y2ps[:])
                if t % 5 == 4:
                    view = vy2a if t < 10 else vy2b
                    c0 = (t - 4) % 10
                    nc.sync.dma_start(
                        out=view[:, c0 : c0 + 5, :], in_=stY["st"][:])

            pass_b(y1_fb, par1, y1_shA, y1_shB, b_sb[0], b1_consumer)

            # ---- layer 2 ----
            allgather_halves(y2_shA, y2_shB, y2_fa, y2_fb)
            pass_a(y2_fa, par2)
            pool_ps = psPool.tile([N_GRAPHS, D], f32)

            def b2_consumer(t, h2):
                nc.tensor.matmul(
                    out=pool_ps[:],
                    lhsT=p_sb[:, t, :],
                    rhs=h2[:],
                    start=(t == 0),
                    stop=(t == TILES - 1),
                    skip_group_check=True,
                )

            pass_b(y2_fb, par2, y2_shA, y2_shB, b_sb[1], b2_consumer)

            # ---- pooled AllReduce ----
            pool_sb = mlp_pool.tile([N_GRAPHS, D], f32)
            nc.vector.tensor_copy(out=pool_sb[:], in_=pool_ps[:])
            nc.sync.dma_start(out=pool_in[:], in_=pool_sb[:])
            nc.gpsimd.collective_compute(
                "AllReduce", mybir.AluOpType.add, replica_groups=rg,
                ins=[pool_in[:]], outs=[pool_out[:]],
            )

            # ---- classifier MLP (replicated, all f32) ----
            pooled = mlp_pool.tile([N_GRAPHS, D], f32)
            nc.sync.dma_start(out=pooled[:], in_=pool_out[:])
            pooledT = mlp_pool.tile([P, KCH, N_GRAPHS], f32)
            for j in range(KCH):
                ps_t = psA.tile([P, N_GRAPHS], f32, tag="psA", name=f"mlp_t_{j}")
                nc.tensor.transpose(
                    out=ps_t[:],
                    in_=pooled[:, j * P : (j + 1) * P],
                    identity=ident[0:N_GRAPHS, 0:N_GRAPHS],
                )
                nc.vector.tensor_copy(out=pooledT[:, j, :], in_=ps_t[:])
            zT = mlp_pool.tile([P, 2, N_GRAPHS], f32)
            for c2 in range(2):
                ps_z = psA.tile([P, N_GRAPHS], f32, tag="psA", name=f"mlp_z_{c2}")
                for j in range(KCH):
                    nc.tensor.matmul(
                        out=ps_z[:],
                        lhsT=wc1_v[:, j, c2, :],
                        rhs=pooledT[:, j, :],
                        start=(j == 0),
                        stop=(j == KCH - 1),
                    )
                nc.scalar.activation(
                    out=zT[:, c2, :], in_=ps_z[:],
                    func=mybir.ActivationFunctionType.Relu,
                    bias=bc1_sb[:, c2 : c2 + 1],
                )
            ps_s = psA.tile([1, N_GRAPHS], f32, tag="psA", name="mlp_s")
            for c2 in range(2):
                nc.tensor.matmul(
                    out=ps_s[:],
                    lhsT=wc2_sb[:, c2 : c2 + 1],
                    rhs=zT[:, c2, :],
                    start=(c2 == 0),
                    stop=(c2 == 1),
                )
            score_sb = mlp_pool.tile([1, N_GRAPHS], f32)
            nc.scalar.activation(
                out=score_sb[:], in_=ps_s[:],
                func=mybir.ActivationFunctionType.Sigmoid,
                bias=bc2_sb[0:1, 0:1],
            )
            nc.sync.dma_start(out=scores[:], in_=score_sb[:])

    nc.finalize()
    return nc


def _wrap_idx(block):
    """[n] -> [16, n/16] wrapped: element i at [i%16, i//16]."""
    n = block.shape[0]
    return block.reshape(n // 16, 16).T


def _prep_inputs(joint_x, joint_edge_index, joint_batch,
                 W_g1, b_g1, W_g2, b_g2, W_c1, b_c1, W_c2, b_c2):
    import heapq

    x = np.asarray(joint_x, np.float32)
    ei = np.asarray(joint_edge_index).astype(np.int64)
    batch = np.asarray(joint_batch).astype(np.int64)
    src, dst = ei[0], ei[1]

    # Unique (src,dst) pairs; multiplicity rides in the S matrix (exact small
    # ints in fp16). Self term (I+A diagonal) is handled separately on-device
    # via a contiguous load of the tile's own y rows, so no self-loop edges.
    pk = src * N_NODES + dst
    upair, mult = np.unique(pk, return_counts=True)
    u_src = upair // N_NODES
    u_dst = upair % N_NODES

    # Rebalance: assign dst nodes to the 160 (core,tile) bins, greedily
    # equalizing per-bin in-edge counts, so every tile needs the same (and
    # minimal) number of 128-edge chunks. The node->position permutation is
    # free to choose: pooling only needs each node's graph id.
    indeg = np.bincount(u_dst, minlength=N_NODES)
    n_bins = N_CORES * TILES
    order = np.argsort(-indeg, kind="stable")
    heap = [(0, b) for b in range(n_bins)]
    heapq.heapify(heap)
    cap = np.full(n_bins, P, np.int64)
    node_bin = np.empty(N_NODES, np.int64)
    node_slot = np.empty(N_NODES, np.int64)
    for n in order:
        while True:
            load, b = heapq.heappop(heap)
            if cap[b] > 0:
                break
        node_bin[n] = b
        node_slot[n] = P - cap[b]
        cap[b] -= 1
        heapq.heappush(heap, (load + int(indeg[n]), b))
    pos = (node_bin // TILES) * PAD_ROWS + (node_bin % TILES) * P + node_slot

    # Gather rows: one per unique (dst-bin, src-half, src) — a single
    # gathered y row feeds every dst slot of that tile that has an edge from
    # src. Rows are split by src HALF (local row </>= 1280) so each tile's
    # first gather calls only depend on the first AllGather half.
    bin_of_pair = node_bin[u_dst]
    src_pos = pos[u_src]
    src_half = (src_pos % PAD_ROWS) // HALF
    src_hidx = (src_pos // PAD_ROWS) * HALF + (src_pos % PAD_ROWS) % HALF
    rk = (bin_of_pair * 2 + src_half) * FULL_HALF + src_hidx
    urow, row_inv = np.unique(rk, return_inverse=True)
    row_bh = urow // FULL_HALF
    row_psrc = urow % FULL_HALF          # index into the half table
    rows_per_bh = np.bincount(row_bh, minlength=n_bins * 2)

    # Equalized per-(tile,half) segment length: max over cores, rounded to 16
    # so the 16-way deal stays inside the segment.
    cnt_cth = rows_per_bh.reshape(N_CORES, TILES, 2)
    n_arr = ((cnt_cth.max(axis=0) + 15) // 16) * 16      # [TILES, 2]
    gh, total_chunks, total_blocks = _derive(n_arr)

    # Global per-chunk lookup tables and per-(tile,half) stream offsets.
    G_t0 = np.empty(total_chunks, np.int64)
    G_blk0 = np.empty(total_chunks, np.int64)            # block id of chunk's first block
    seg_off_glob = np.empty((TILES, 2), np.int64)        # global row offset of segment
    for (g, h), info in gh.items():
        co, bo = info["chunk_off"], info["block_off"]
        K = info["K"]
        G_t0[co : co + K] = info["chunk_t0"]
        G_blk0[co : co + K] = bo + info["block_base"][:-1]
        for tt in range(GRP):
            seg_off_glob[g * GRP + tt, h] = co * P + info["seg_off"][tt]

    # Rank within (bin, half) (urow sorted => grouped, ascending src pos),
    # then deal 16 ways within the segment so each SDMA engine (descriptor
    # i -> engine i%16) walks ascending HBM addresses.
    bh_starts = np.concatenate([[0], np.cumsum(rows_per_bh)])
    row_rank = np.arange(len(urow)) - bh_starts[row_bh]
    row_bin = row_bh // 2
    row_h = row_bh % 2
    row_t = row_bin % TILES
    seg_n = n_arr[row_t, row_h]
    sub_len = seg_n // 16
    deal_pos = (row_rank % sub_len) * 16 + row_rank // sub_len
    row_gpos = seg_off_glob[row_t, row_h] + deal_pos     # global stream row
    row_chunk = row_gpos // P
    row_e = row_gpos % P
    row_blk = G_blk0[row_chunk] + (row_t % GRP) - G_t0[row_chunk]

    per_core = []
    pair_slot = node_slot[u_dst]
    pair_blk = row_blk[row_inv]
    pair_e = row_e[row_inv]
    pair_core = bin_of_pair // TILES
    for c in range(N_CORES):
        m = row_bin // TILES == c
        idx_flat = np.zeros(total_chunks * P, np.int16)
        idx_flat[row_gpos[m]] = row_psrc[m].astype(np.int16)
        # S packed [128 partitions, total_blocks*128]
        S = np.zeros((P, total_blocks * P), F16)
        pm = pair_core == c
        S[pair_e[pm], pair_blk[pm] * P + pair_slot[pm]] = mult[pm]
        per_core.append((idx_flat, S))

    # node at each padded position (for x shard + pooling construction)
    node_at = np.full(N_CORES * PAD_ROWS, -1, np.int64)
    node_at[pos] = np.arange(N_NODES)

    in_maps = []
    w1_pack = np.ascontiguousarray(
        W_g1.astype(F16).reshape(KCH, P, D).transpose(1, 0, 2).reshape(P, KCH * D))
    w2_pack = np.ascontiguousarray(
        W_g2.astype(F16).reshape(KCH, P, D).transpose(1, 0, 2).reshape(P, KCH * D))
    b1_pack = np.ascontiguousarray(np.broadcast_to(
        np.asarray(b_g1, np.float32), (P, D)))
    b2_pack = np.ascontiguousarray(np.broadcast_to(
        np.asarray(b_g2, np.float32), (P, D)))
    wc1_pack = np.ascontiguousarray(
        np.asarray(W_c1, np.float32).reshape(KCH, P, 2, P)
        .transpose(1, 0, 2, 3).reshape(P, KCH * 2 * P))
    bc1_pack = np.ascontiguousarray(np.asarray(b_c1, np.float32).reshape(2, P).T)
    wc2_pack = np.ascontiguousarray(np.asarray(W_c2, np.float32).reshape(2, P).T)
    bc2_pack = np.asarray(b_c2, np.float32).reshape(1, 1)

    x_bf = x.astype(F16)
    for c in range(N_CORES):
        idx_flat, s_pack = per_core[c]

        # x shard in permuted position space
        nodes_c = node_at[c * PAD_ROWS : (c + 1) * PAD_ROWS]
        real = nodes_c >= 0
        xs = np.zeros((PAD_ROWS, D), F16)
        xs[real] = x_bf[nodes_c[real]]

        # gather idx table [128, total_chunks*8] wrapped per call
        cols = []
        for g in range(N_GRP):
            for h in range(2):
                info = gh[(g, h)]
                co = info["chunk_off"]
                k = 0
                for sz in info["sizes"]:
                    block = idx_flat[(co + k) * P : (co + k + sz) * P]
                    cols.append(_wrap_idx(block))
                    k += sz
        idx16 = np.concatenate(cols, axis=1)          # [16, total_chunks*8]
        idx_pack = np.ascontiguousarray(np.tile(idx16, (8, 1)))

        # pooling one-hot [128, TILES*64]
        Pm = np.zeros((PAD_ROWS, N_GRAPHS), F16)
        Pm[real, batch[nodes_c[real]]] = 1
        p_pack = np.ascontiguousarray(
            Pm.reshape(TILES, P, N_GRAPHS).transpose(1, 0, 2).reshape(P, -1))

        in_maps.append({
            "x_sh": xs,
            "idx_all": idx_pack,
            "s_all": s_pack,
            "p_all": p_pack,
            "w1": w1_pack, "w2": w2_pack,
            "b1b": b1_pack, "b2b": b2_pack,
            "wc1": wc1_pack, "bc1": bc1_pack,
            "wc2": wc2_pack, "bc2": bc2_pack,
        })
    return tuple(int(v) for v in n_arr.flatten()), in_maps


def kernel(**inputs):
    global LAST_EXEC_NS, LAST_RESULTS
    kk, in_maps = _prep_inputs(**inputs)
    if kk not in _prog_cache:
        _prog_cache[kk] = _build_program(kk)
    nc = _prog_cache[kk]
    trace = os.environ.get("GNN_TRACE", "0") == "1"
    res = run_bass_kernel_spmd(
        nc, in_maps, core_ids=list(range(N_CORES)), trace=trace,
        tmpdir=os.environ.get("GNN_TRACE_DIR") or None,
    )
    LAST_EXEC_NS = getattr(res, "exec_time_ns", None)
    LAST_RESULTS = res
    return np.asarray(res.results[0]["scores"]).reshape(N_GRAPHS).astype(np.float32)



# revision 8
# speedup vs baseline: 7.4593x; 7.4593x over previous
"""CrossEncoderGNN (2x GIN layer + sum-pool + MLP + sigmoid) on 8 trn2 NeuronCores.

Strategy
--------
The network is LINEAR at node level (no activation inside the GIN layers;
relu/sigmoid only appear after graph pooling).  With A the edge-multiplicity
adjacency (agg = A h), B the [N, G] node->graph one-hot, the pooled vector
collapses algebraically:

  pooled = B^T (I+A) ((I+A) x W1 + 1 b1^T) W2 + 1 b2^T summed per graph
         = v^T x W1 W2 + s (W2^T b1)^T + cnt b2^T

where v = ((I+A)^2)^T B is a small INTEGER matrix [N, G] computed on host
from the edge list + batch vector (graph-structure preprocessing, same
category as the baseline's one-hot scatter matrices), s = u^T 1 with
u = (I+A)^T B, and cnt = nodes per graph.

Device work per core (row shard of 2500 nodes, padded to 2560 = 20 tiles):
  QT_c = x_c^T v_c           [512, 64]  (80 small f16 matmuls, x as lhsT)
  AllReduce QT (131 KB f32)  -> QT on every core
  RT  = W1^T QT              (16 f32 matmuls)
  PT  = W2^T RT + C          (C = outer(W2^T b1, s) + outer(b2, cnt), host)
  zT  = relu(Wc1^T PT + bc1) (8 matmuls + activation)
  score = sigmoid(Wc2^T zT + bc2) -> [1, 64]

Everything that touches joint_x runs on device; host prep is integer graph
structure + weight repacking only.
"""

import sys

for _p in ("/opt/trn_rl_repo", "/root/.axon_site/_ro/trn_rl_repo"):
    if _p not in sys.path:
        sys.path.insert(0, _p)

import os
import numpy as np

import concourse.bass as bass
import concourse.bacc as bacc
import concourse.tile as tile
from concourse import mybir
from concourse.bass_utils import run_bass_kernel_spmd

F16 = np.float16

N_NODES = 20000
D = 512
G = 64
N_CORES = 8
P = 128
ROWS = N_NODES // N_CORES          # 2500
TILES = (ROWS + P - 1) // P        # 20
PAD_ROWS = TILES * P               # 2560
KCH = D // P                       # 4

LAST_EXEC_NS = None
LAST_RESULTS = None

_prog_cache = {}


def _build_program():
    f32 = mybir.dt.float32
    f16 = mybir.dt.float16

    nc = bacc.Bacc("TRN2", debug=False, num_devices=N_CORES, num_swdge_queues=2)

    # ---- I/O ----
    x_in = nc.dram_tensor("x_sh", [P, TILES * D], f16, kind="ExternalInput")
    v_in = nc.dram_tensor("v_sh", [P, TILES * G], f16, kind="ExternalInput")
    w1_in = nc.dram_tensor("w1", [P, KCH * KCH * P], f32, kind="ExternalInput")
    w2_in = nc.dram_tensor("w2", [P, KCH * KCH * P], f32, kind="ExternalInput")
    wc1_in = nc.dram_tensor("wc1", [P, KCH * 2 * P], f32, kind="ExternalInput")
    wc2_in = nc.dram_tensor("wc2", [P, 2], f32, kind="ExternalInput")
    cbias_in = nc.dram_tensor("cbias", [P, KCH * G], f32, kind="ExternalInput")
    bc1_in = nc.dram_tensor("bc1", [P, 2], f32, kind="ExternalInput")
    bc2_in = nc.dram_tensor("bc2", [1, 1], f32, kind="ExternalInput")
    scores = nc.dram_tensor("scores", [1, G], f32, kind="ExternalOutput")

    # ---- internal DRAM for the collective ----
    qpart = nc.dram_tensor("qpart", [P, KCH * G], f32)
    qfull = nc.dram_tensor("qfull", [P, KCH * G], f32, addr_space="Shared")

    rg = [list(range(N_CORES))]

    with tile.TileContext(nc) as tc:
        with (
            tc.tile_pool(name="const", bufs=1) as const,
            tc.tile_pool(name="xin", bufs=1) as xin,
            tc.tile_pool(name="work", bufs=1) as work,
            tc.tile_pool(name="ps", bufs=4, space="PSUM") as ps,
        ):
            # x shard: 4 chunked DMAs (5 tiles each) so matmuls start after
            # the first quarter lands; SP HWDGE ring.
            x_sb = xin.tile([P, TILES * D], f16)
            CH = 5 * D
            for q in range(4):
                nc.sync.dma_start(
                    out=x_sb[:, q * CH : (q + 1) * CH],
                    in_=x_in[:, q * CH : (q + 1) * CH],
                )
            v_sb = const.tile([P, TILES * G], f16)
            nc.sync.dma_start(out=v_sb[:], in_=v_in[:])

            # weights on the ACT ring, overlapping the pool matmuls
            w1_sb = const.tile([P, KCH * KCH * P], f32)
            nc.scalar.dma_start(out=w1_sb[:], in_=w1_in[:])
            w2_sb = const.tile([P, KCH * KCH * P], f32)
            nc.scalar.dma_start(out=w2_sb[:], in_=w2_in[:])
            wc1_sb = const.tile([P, KCH * 2 * P], f32)
            nc.scalar.dma_start(out=wc1_sb[:], in_=wc1_in[:])
            wc2_sb = const.tile([P, 2], f32)
            nc.scalar.dma_start(out=wc2_sb[:], in_=wc2_in[:])
            c_sb = const.tile([P, KCH * G], f32)
            nc.scalar.dma_start(out=c_sb[:], in_=cbias_in[:])
            bc1_sb = const.tile([P, 2], f32)
            nc.scalar.dma_start(out=bc1_sb[:], in_=bc1_in[:])
            bc2_sb = const.tile([1, 1], f32)
            nc.scalar.dma_start(out=bc2_sb[:], in_=bc2_in[:])

            xv = x_sb[:].rearrange("p (t d) -> p t d", d=D)
            vv = v_sb[:].rearrange("p (t g) -> p t g", g=G)

            # ---- QT = x^T v, accumulated over the 20 node tiles ----
            # One logical accumulation group for the whole bank: start=True
            # zeroes the full 2KB PSUM zero region, so only the very first
            # matmul may carry it; the other regions' first writes land in
            # the freshly-cleared region and accumulate from there.
            psQ = ps.tile([P, KCH, G], f32, tag="ps", name="psQ")
            for t in range(TILES):
                for j in range(KCH):
                    nc.tensor.matmul(
                        out=psQ[:, j, :],
                        lhsT=xv[:, t, j * P : (j + 1) * P],
                        rhs=vv[:, t, :],
                        start=(t == 0 and j == 0),
                        stop=(t == TILES - 1 and j == KCH - 1),
                        skip_group_check=True,
                    )
            q_sb = work.tile([P, KCH, G], f32)
            nc.vector.tensor_copy(out=q_sb[:], in_=psQ[:])
            nc.sync.dma_start(
                out=qpart.ap().rearrange("p (j g) -> p j g", g=G), in_=q_sb[:]
            )

            nc.gpsimd.collective_compute(
                "AllReduce", mybir.AluOpType.add, replica_groups=rg,
                ins=[qpart[:]], outs=[qfull[:]],
            )

            qt = work.tile([P, KCH, G], f32)
            nc.sync.dma_start(
                out=qt[:], in_=qfull.ap().rearrange("p (j g) -> p j g", g=G)
            )

            # ---- RT = W1^T QT ----
            w1v = w1_sb[:].rearrange("p (j c q) -> p j c q", c=KCH, q=P)
            psR = ps.tile([P, KCH, G], f32, tag="ps", name="psR")
            for c in range(KCH):
                for j in range(KCH):
                    nc.tensor.matmul(
                        out=psR[:, c, :],
                        lhsT=w1v[:, j, c, :],
                        rhs=qt[:, j, :],
                        start=(c == 0 and j == 0),
                        stop=(c == KCH - 1 and j == KCH - 1),
                        skip_group_check=True,
                    )
            rt = work.tile([P, KCH, G], f32)
            nc.vector.tensor_copy(out=rt[:], in_=psR[:])

            # ---- PT = W2^T RT + C ----
            w2v = w2_sb[:].rearrange("p (j c q) -> p j c q", c=KCH, q=P)
            psP = ps.tile([P, KCH, G], f32, tag="ps", name="psP")
            for c in range(KCH):
                for j in range(KCH):
                    nc.tensor.matmul(
                        out=psP[:, c, :],
                        lhsT=w2v[:, j, c, :],
                        rhs=rt[:, j, :],
                        start=(c == 0 and j == 0),
                        stop=(c == KCH - 1 and j == KCH - 1),
                        skip_group_check=True,
                    )
            pt = work.tile([P, KCH, G], f32)
            nc.vector.tensor_add(
                out=pt[:], in0=psP[:],
                in1=c_sb[:].rearrange("p (j g) -> p j g", g=G),
            )

            # ---- zT = relu(Wc1^T PT + bc1) ----
            wc1v = wc1_sb[:].rearrange("p (j c q) -> p j c q", c=2, q=P)
            zt = work.tile([P, 2, G], f32)
            for c2 in range(2):
                psZ = ps.tile([P, G], f32, tag="ps", name=f"psZ{c2}")
                for j in range(KCH):
                    nc.tensor.matmul(
                        out=psZ[:],
                        lhsT=wc1v[:, j, c2, :],
                        rhs=pt[:, j, :],
                        start=(j == 0),
                        stop=(j == KCH - 1),
                    )
                nc.scalar.activation(
                    out=zt[:, c2, :], in_=psZ[:],
                    func=mybir.ActivationFunctionType.Relu,
                    bias=bc1_sb[:, c2 : c2 + 1],
                )

            # ---- score = sigmoid(Wc2^T zT + bc2) ----
            psS = ps.tile([1, G], f32, tag="ps", name="psS")
            for c2 in range(2):
                nc.tensor.matmul(
                    out=psS[:],
                    lhsT=wc2_sb[:, c2 : c2 + 1],
                    rhs=zt[:, c2, :],
                    start=(c2 == 0),
                    stop=(c2 == 1),
                )
            sc = work.tile([1, G], f32)
            nc.scalar.activation(
                out=sc[:], in_=psS[:],
                func=mybir.ActivationFunctionType.Sigmoid,
                bias=bc2_sb[0:1, 0:1],
            )
            nc.sync.dma_start(out=scores[:], in_=sc[:])

    nc.finalize()
    return nc


def _prep_inputs(joint_x, joint_edge_index, joint_batch,
                 W_g1, b_g1, W_g2, b_g2, W_c1, b_c1, W_c2, b_c2):
    x = np.asarray(joint_x, np.float32)
    ei = np.asarray(joint_edge_index).astype(np.int64)
    batch = np.asarray(joint_batch).astype(np.int64)
    src, dst = ei[0], ei[1]

    # u = (I+A)^T B : u[n,g] = [batch[n]==g] + #edges n->m with batch[m]==g
    u = np.bincount(src * G + batch[dst], minlength=N_NODES * G)
    u = u.reshape(N_NODES, G).astype(np.float64)
    u[np.arange(N_NODES), batch] += 1.0

    # v = (I+A)^T u : v[n,g] = u[n,g] + sum over out-edges n->m of u[m,g]
    order = np.argsort(src, kind="stable")
    ssrc = src[order]
    udst = u[dst[order]]
    bounds = np.minimum(
        np.searchsorted(ssrc, np.arange(N_NODES)), max(len(ssrc) - 1, 0)
    )
    v = u.copy()
    if len(ssrc):
        seg = np.add.reduceat(udst, bounds, axis=0)
        has = np.zeros(N_NODES, bool)
        has[ssrc] = True
        v[has] += seg[has]

    s = u.sum(axis=0)                                     # [G]
    cnt = np.bincount(batch, minlength=G).astype(np.float64)

    W1 = np.asarray(W_g1, np.float64)
    W2 = np.asarray(W_g2, np.float64)
    bb = W2.T @ np.asarray(b_g1, np.float64)              # [512]
    C = np.outer(bb, s) + np.outer(np.asarray(b_g2, np.float64), cnt)

    def pack_w(W, cdim):
        return np.ascontiguousarray(
            np.asarray(W, np.float32).reshape(KCH, P, cdim, P)
            .transpose(1, 0, 2, 3).reshape(P, -1))

    w1_pack = pack_w(W1, KCH)
    w2_pack = pack_w(W2, KCH)
    wc1_pack = pack_w(np.asarray(W_c1, np.float32), 2)
    wc2_pack = np.ascontiguousarray(
        np.asarray(W_c2, np.float32).reshape(2, P).T)
    c_pack = np.ascontiguousarray(
        C.astype(np.float32).reshape(KCH, P, G).transpose(1, 0, 2).reshape(P, -1))
    bc1_pack = np.ascontiguousarray(np.asarray(b_c1, np.float32).reshape(2, P).T)
    bc2_pack = np.asarray(b_c2, np.float32).reshape(1, 1)

    x16 = x.astype(F16)
    v16 = v.astype(F16)
    in_maps = []
    for c in range(N_CORES):
        lo, hi = c * ROWS, (c + 1) * ROWS
        xs = np.zeros((TILES, P, D), F16)
        xs.reshape(-1, D)[:ROWS] = x16[lo:hi]
        vs = np.zeros((TILES, P, G), F16)
        vs.reshape(-1, G)[:ROWS] = v16[lo:hi]
        in_maps.append({
            "x_sh": np.ascontiguousarray(
                xs.transpose(1, 0, 2).reshape(P, TILES * D)),
            "v_sh": np.ascontiguousarray(
                vs.transpose(1, 0, 2).reshape(P, TILES * G)),
            "w1": w1_pack, "w2": w2_pack,
            "wc1": wc1_pack, "wc2": wc2_pack,
            "cbias": c_pack, "bc1": bc1_pack, "bc2": bc2_pack,
        })
    return in_maps


def kernel(**inputs):
    global LAST_EXEC_NS, LAST_RESULTS
    in_maps = _prep_inputs(**inputs)
    if "prog" not in _prog_cache:
        _prog_cache["prog"] = _build_program()
    nc = _prog_cache["prog"]
    trace = os.environ.get("GNN_TRACE", "0") == "1"
    res = run_bass_kernel_spmd(
        nc, in_maps, core_ids=list(range(N_CORES)), trace=trace,
        tmpdir=os.environ.get("GNN_TRACE_DIR") or None,
    )
    LAST_EXEC_NS = getattr(res, "exec_time_ns", None)
    LAST_RESULTS = res
    return np.asarray(res.results[0]["scores"]).reshape(G).astype(np.float32)
